# revision 3
# baseline (speedup 1.0000x reference)
"""Trainium2 Bass kernel for nn_BahdanauAttention_17257178595902.

Reference math: softmax over a size-1 axis makes alpha identically 1.0,
so context[b, :] = sum_t a[b, t, :] and alpha = ones(B, Tx). The kernel
is a pure memory-bound reduction over `a` (128 x 400 x 512 f32, ~105 MB).

Sharding: data-parallel over batch B across 8 cores (16 rows each).
Per core the shard a[16, 400, 512] is viewed as [128, 50, 512]: SBUF
partition p holds 50 consecutive Tx-rows, all belonging to local batch
b = p // 8. The Tx-sum is a single PE accumulation group: a constant
block-diagonal ones matrix S[128, 16] (S[p, p//8] = 1) is the stationary
operand, and each [128, 512] slice of the shard streams through as the
moving operand, accumulating context[16, 512] in one PSUM bank.
alpha is a memset-ones tile DMA'd out.
"""

import sys

for _p in ("/opt/trn_rl_repo",):
    if _p not in sys.path:
        sys.path.append(_p)

import numpy as np

B, TX, ENC = 128, 400, 512
NCORES = 8
BS = B // NCORES  # 16 local batch rows per core
P = 128  # SBUF partitions
TPP = BS * TX // P  # 50 Tx-rows per partition
TCH = 10  # Tx-rows per DMA chunk (per partition)
NCH = TPP // TCH  # 5 chunks of [128, 10, 512] = 2.62 MB each

TRACE = False  # set by test harness to capture an NTFF profile
LAST_RESULTS = None  # BassKernelResults of the most recent run

_CACHE = {}


def _build_nc():
    import concourse.bacc as bacc
    import concourse.mybir as mybir
    from concourse.tile import TileContext

    f32 = mybir.dt.float32
    nc = bacc.Bacc("TRN2", target_bir_lowering=False)

    a = nc.dram_tensor("a_shard", [P, TPP, ENC], f32, kind="ExternalInput")
    s = nc.dram_tensor("ones_blk", [P, BS], f32, kind="ExternalInput")
    ctx_o = nc.dram_tensor("ctx_out", [BS, ENC], f32, kind="ExternalOutput")
    alpha_o = nc.dram_tensor("alpha_out", [BS, TX], f32, kind="ExternalOutput")

    with TileContext(nc) as tc:
        with (
            tc.tile_pool(name="io", bufs=3) as pool,
            tc.tile_pool(name="cst", bufs=1) as cpool,
            tc.tile_pool(name="acc", bufs=1, space="PSUM") as ppool,
            tc.tile_pool(name="outp", bufs=1) as opool,
        ):
            S = cpool.tile([P, BS], f32)
            nc.sync.dma_start(S[:], s[:])

            alpha_t = opool.tile([BS, TX], f32)
            nc.vector.memset(alpha_t[:], 1.0)
            nc.sync.dma_start(alpha_o[:], alpha_t[:])

            acc = ppool.tile([BS, ENC], f32)
            for c in range(NCH):
                tl = pool.tile([P, TCH, ENC], f32)
                nc.sync.dma_start(tl[:], a[:, c * TCH : (c + 1) * TCH, :])
                for t in range(TCH):
                    nc.tensor.matmul(
                        acc[:],
                        S[:],
                        tl[:, t, :],
                        start=(c == 0 and t == 0),
                        stop=(c == NCH - 1 and t == TCH - 1),
                    )

            out_sb = opool.tile([BS, ENC], f32)
            nc.vector.tensor_copy(out_sb[:], acc[:])
            nc.sync.dma_start(ctx_o[:], out_sb[:])

    nc.finalize()
    return nc


def _ones_blk():
    s = np.zeros((P, BS), dtype=np.float32)
    s[np.arange(P), np.arange(P) // (P // BS)] = 1.0
    return s


def kernel(a, h, coverage, Wmat, v, w, b):
    global LAST_RESULTS
    from concourse.bass_utils import run_bass_kernel_spmd

    if "nc" not in _CACHE:
        _CACHE["nc"] = _build_nc()
    nc = _CACHE["nc"]

    a = np.ascontiguousarray(np.asarray(a, dtype=np.float32))
    s = _ones_blk()
    in_maps = [
        {
            "a_shard": a[c * BS : (c + 1) * BS].reshape(P, TPP, ENC),
            "ones_blk": s,
        }
        for c in range(NCORES)
    ]

    res = run_bass_kernel_spmd(nc, in_maps, core_ids=list(range(NCORES)), trace=TRACE)
    LAST_RESULTS = res

    context = np.concatenate([r["ctx_out"] for r in res.results], axis=0)
    alpha = np.concatenate([r["alpha_out"] for r in res.results], axis=0)
    return context[:, None, :], alpha


# revision 4
# speedup vs baseline: 2.2553x; 2.2553x over previous
"""Trainium2 Bass kernel for nn_BahdanauAttention_17257178595902.

Reference math: softmax over a size-1 axis makes alpha identically 1.0,
so context[b, :] = sum_t a[b, t, :] and alpha = ones(B, Tx). The kernel
is a pure memory-bound reduction over `a` (128 x 400 x 512 f32, ~105 MB).

Sharding: data-parallel over batch B across 8 cores (16 rows each).
Per core the shard a[16, 400, 512] is viewed as [128, 50, 512]: SBUF
partition p holds 50 consecutive Tx-rows, all of local batch b = p // 8.

Per-core schedule (tuned against the CoreSim cost model, 59.4us -> 26.3us):
 - 11 uneven chunks (small first to warm PE early, small last to cut the
   tail), loads striped over the three DMA issue paths (SP HWDGE, ACT
   HWDGE, GpSimd SWDGE) and all issued upfront (bufs = n_chunks, ~132 KB
   per partition, no buffer stalls).
 - the Tx-sum is split three ways: PE (fp32 matmuls with a constant
   block-diagonal ones stationary S[128,16], accumulated in one PSUM
   group), DVE (strided tensor_reduce over t + adds into dve_acc), and
   GpSimd (tensor_add chain into pool_acc).
 - merge: pool_acc folded into dve_acc on DVE, then one final PE matmul
   closes the PSUM accumulation group; ACT copies PSUM->SBUF and DMAs
   the context out. alpha is a GpSimd memset-ones tile DMA'd out on the
   idle SP ring.
"""

import sys

for _p in ("/opt/trn_rl_repo",):
    if _p not in sys.path:
        sys.path.append(_p)

import numpy as np

B, TX, ENC = 128, 400, 512
NCORES = 8
BS = B // NCORES  # 16 local batch rows per core
P = 128  # SBUF partitions
TPP = BS * TX // P  # 50 Tx-rows per partition

# chunk sizes (t-rows), per-chunk (pe_k, pool_k) slice split (rest -> DVE),
# and DMA ring per chunk (0 = sync/SP HWDGE, 1 = scalar/ACT HWDGE, 2 = SWDGE)
CHUNKS = [2, 4, 5, 6, 6, 6, 6, 6, 5, 2, 2]
SPLIT = [(1, 1), (1, 1), (1, 2), (2, 2), (2, 2), (1, 2), (2, 2), (2, 2), (1, 2), (1, 1), (1, 1)]
RING = [0, 1, 2, 0, 1, 2, 0, 1, 2, 0, 1]

TRACE = False  # set by test harness to capture an NTFF profile
LAST_RESULTS = None  # BassKernelResults of the most recent run

_CACHE = {}


def _build_nc():
    import concourse.bacc as bacc
    import concourse.mybir as mybir
    from concourse.tile import TileContext

    f32 = mybir.dt.float32
    nc = bacc.Bacc("TRN2", target_bir_lowering=False)

    a = nc.dram_tensor("a_shard", [P, TPP, ENC], f32, kind="ExternalInput")
    s = nc.dram_tensor("ones_blk", [P, BS], f32, kind="ExternalInput")
    ctx_o = nc.dram_tensor("ctx_out", [BS, ENC], f32, kind="ExternalOutput")
    alpha_o = nc.dram_tensor("alpha_out", [BS, TX], f32, kind="ExternalOutput")

    with TileContext(nc) as tc:
        rings = [nc.sync, nc.scalar, nc.gpsimd]
        with (
            tc.tile_pool(name="io", bufs=len(CHUNKS)) as pool,
            tc.tile_pool(name="cst", bufs=1) as cpool,
            tc.tile_pool(name="red", bufs=3) as rpool,
            tc.tile_pool(name="accp", bufs=1) as apool,
            tc.tile_pool(name="psum", bufs=1, space="PSUM") as ppool,
            tc.tile_pool(name="outp", bufs=1) as opool,
        ):
            S = cpool.tile([P, BS], f32)
            nc.scalar.dma_start(S[:], s[:])

            # all chunk loads upfront; per-ring FIFO order = chunk order
            tiles = []
            t0 = 0
            maxc = max(CHUNKS)
            for c, tch in enumerate(CHUNKS):
                tl = pool.tile([P, maxc, ENC], f32, tag="io", name=f"io{c}")
                rings[RING[c]].dma_start(tl[:, :tch, :], a[:, t0 : t0 + tch, :])
                tiles.append(tl)
                t0 += tch

            pacc = ppool.tile([BS, ENC], f32)
            dve_acc = apool.tile([P, ENC], f32, name="dve_acc")
            pool_acc = apool.tile([P, ENC], f32, name="pool_acc")

            mm_i = 0
            dve_chunks = 0
            pool_slices = 0
            for c, (tch, (pe_k, pool_k)) in enumerate(zip(CHUNKS, SPLIT)):
                tl = tiles[c]
                for t in range(pe_k):
                    nc.tensor.matmul(
                        pacc[:], S[:], tl[:, t, :], start=(mm_i == 0), stop=False
                    )
                    mm_i += 1
                for t in range(pe_k, pe_k + pool_k):
                    if pool_slices == 0:
                        nc.gpsimd.tensor_copy(pool_acc[:], tl[:, t, :])
                    else:
                        nc.gpsimd.tensor_add(pool_acc[:], pool_acc[:], tl[:, t, :])
                    pool_slices += 1
                if tch - pe_k - pool_k > 0:
                    tview = tl[:, pe_k + pool_k : tch, :].rearrange("p t e -> p e t")
                    if dve_chunks == 0:
                        nc.vector.tensor_reduce(
                            dve_acc[:], tview,
                            axis=mybir.AxisListType.X, op=mybir.AluOpType.add,
                        )
                    else:
                        part = rpool.tile([P, ENC], f32, tag="part", name="part")
                        nc.vector.tensor_reduce(
                            part[:], tview,
                            axis=mybir.AxisListType.X, op=mybir.AluOpType.add,
                        )
                        nc.vector.tensor_add(dve_acc[:], dve_acc[:], part[:])
                    dve_chunks += 1

            # alpha: memset rides Pool after its add chain; out on idle SP ring
            alpha_t = opool.tile([BS, TX], f32)
            nc.gpsimd.memset(alpha_t[:], 1.0)
            nc.sync.dma_start(alpha_o[:], alpha_t[:])

            # merge partial accumulators, close the PSUM group, write out
            nc.vector.tensor_add(dve_acc[:], dve_acc[:], pool_acc[:])
            nc.tensor.matmul(pacc[:], S[:], dve_acc[:], start=(mm_i == 0), stop=True)

            out_sb = opool.tile([BS, ENC], f32)
            nc.scalar.copy(out_sb[:], pacc[:])
            nc.scalar.dma_start(ctx_o[:], out_sb[:])

    nc.finalize()
    return nc


def _ones_blk():
    s = np.zeros((P, BS), dtype=np.float32)
    s[np.arange(P), np.arange(P) // (P // BS)] = 1.0
    return s


def kernel(a, h, coverage, Wmat, v, w, b):
    global LAST_RESULTS
    from concourse.bass_utils import run_bass_kernel_spmd

    if "nc" not in _CACHE:
        _CACHE["nc"] = _build_nc()
    nc = _CACHE["nc"]

    a = np.ascontiguousarray(np.asarray(a, dtype=np.float32))
    s = _ones_blk()
    in_maps = [
        {
            "a_shard": a[c * BS : (c + 1) * BS].reshape(P, TPP, ENC),
            "ones_blk": s,
        }
        for c in range(NCORES)
    ]

    res = run_bass_kernel_spmd(nc, in_maps, core_ids=list(range(NCORES)), trace=TRACE)
    LAST_RESULTS = res

    context = np.concatenate([r["ctx_out"] for r in res.results], axis=0)
    alpha = np.concatenate([r["alpha_out"] for r in res.results], axis=0)
    return context[:, None, :], alpha


# revision 5
# speedup vs baseline: 2.2784x; 1.0102x over previous
"""Trainium2 Bass kernel for nn_BahdanauAttention_17257178595902.

Reference math: softmax over a size-1 axis makes alpha identically 1.0,
so context[b, :] = sum_t a[b, t, :] and alpha = ones(B, Tx). The kernel
is a pure memory-bound reduction over `a` (128 x 400 x 512 f32, ~105 MB).

Sharding: data-parallel over batch B across 8 cores (16 rows each).
Per core the shard a[16, 400, 512] is viewed as [128, 50, 512]: SBUF
partition p holds 50 consecutive Tx-rows, all of local batch b = p // 8.

Per-core schedule (tuned against the CoreSim cost model, 59.4us -> 26.3us):
 - 11 uneven chunks (small first to warm PE early, small last to cut the
   tail), loads striped over the three DMA issue paths (SP HWDGE, ACT
   HWDGE, GpSimd SWDGE) and all issued upfront (bufs = n_chunks, ~132 KB
   per partition, no buffer stalls).
 - the Tx-sum is split three ways: PE (fp32 matmuls with a constant
   block-diagonal ones stationary S[128,16], accumulated in one PSUM
   group), DVE (strided tensor_reduce over t + adds into dve_acc), and
   GpSimd (tensor_add chain into pool_acc).
 - merge: pool_acc folded into dve_acc on DVE, then one final PE matmul
   closes the PSUM accumulation group; ACT copies PSUM->SBUF and DMAs
   the context out. alpha is a GpSimd memset-ones tile DMA'd out on the
   idle SP ring.
"""

import sys

for _p in ("/opt/trn_rl_repo",):
    if _p not in sys.path:
        sys.path.append(_p)

import numpy as np

B, TX, ENC = 128, 400, 512
NCORES = 8
BS = B // NCORES  # 16 local batch rows per core
P = 128  # SBUF partitions
TPP = BS * TX // P  # 50 Tx-rows per partition

# chunk sizes (t-rows), per-chunk (pe_k, pool_k) slice split (rest -> DVE),
# and DMA ring per chunk (0 = sync/SP HWDGE, 1 = scalar/ACT HWDGE, 2 = SWDGE)
CHUNKS = [2, 4, 5, 6, 6, 6, 6, 6, 5, 2, 2]
SPLIT = [(1, 1), (1, 1), (1, 2), (2, 2), (2, 2), (1, 2), (2, 2), (2, 2), (1, 2), (1, 1), (1, 1)]
RING = [0, 1, 2, 0, 1, 2, 0, 1, 2, 0, 1]

TRACE = False  # set by test harness to capture an NTFF profile
LAST_RESULTS = None  # BassKernelResults of the most recent run

_CACHE = {}


def _build_nc():
    import concourse.bacc as bacc
    import concourse.mybir as mybir
    from concourse.tile import TileContext

    f32 = mybir.dt.float32
    nc = bacc.Bacc("TRN2", target_bir_lowering=False)

    a = nc.dram_tensor("a_shard", [P, TPP, ENC], f32, kind="ExternalInput")
    s = nc.dram_tensor("ones_blk", [P, BS], f32, kind="ExternalInput")
    ctx_o = nc.dram_tensor("ctx_out", [BS, ENC], f32, kind="ExternalOutput")
    alpha_o = nc.dram_tensor("alpha_out", [BS, TX], f32, kind="ExternalOutput")

    with TileContext(nc) as tc:
        rings = [nc.sync, nc.scalar, nc.gpsimd]
        with (
            tc.tile_pool(name="io", bufs=len(CHUNKS)) as pool,
            tc.tile_pool(name="cst", bufs=1) as cpool,
            tc.tile_pool(name="red", bufs=3) as rpool,
            tc.tile_pool(name="accp", bufs=1) as apool,
            tc.tile_pool(name="psum", bufs=1, space="PSUM") as ppool,
            tc.tile_pool(name="outp", bufs=1) as opool,
        ):
            S = cpool.tile([P, BS], f32)
            nc.scalar.dma_start(S[:], s[:])

            # all chunk loads upfront; per-ring FIFO order = chunk order
            tiles = []
            t0 = 0
            maxc = max(CHUNKS)
            for c, tch in enumerate(CHUNKS):
                tl = pool.tile([P, maxc, ENC], f32, tag="io", name=f"io{c}")
                rings[RING[c]].dma_start(tl[:, :tch, :], a[:, t0 : t0 + tch, :])
                tiles.append(tl)
                t0 += tch

            pacc = ppool.tile([BS, ENC], f32)
            dve_acc = apool.tile([P, ENC], f32, name="dve_acc")
            pool_acc = apool.tile([P, ENC], f32, name="pool_acc")

            mm_i = 0
            dve_chunks = 0
            pool_slices = 0
            for c, (tch, (pe_k, pool_k)) in enumerate(zip(CHUNKS, SPLIT)):
                tl = tiles[c]
                for t in range(pe_k):
                    nc.tensor.matmul(
                        pacc[:], S[:], tl[:, t, :], start=(mm_i == 0), stop=False
                    )
                    mm_i += 1
                for t in range(pe_k, pe_k + pool_k):
                    if pool_slices == 0:
                        nc.gpsimd.tensor_copy(pool_acc[:], tl[:, t, :])
                    else:
                        nc.gpsimd.tensor_add(pool_acc[:], pool_acc[:], tl[:, t, :])
                    pool_slices += 1
                if tch - pe_k - pool_k > 0:
                    tview = tl[:, pe_k + pool_k : tch, :].rearrange("p t e -> p e t")
                    if dve_chunks == 0:
                        nc.vector.tensor_reduce(
                            dve_acc[:], tview,
                            axis=mybir.AxisListType.X, op=mybir.AluOpType.add,
                        )
                    else:
                        part = rpool.tile([P, ENC], f32, tag="part", name="part")
                        nc.vector.tensor_reduce(
                            part[:], tview,
                            axis=mybir.AxisListType.X, op=mybir.AluOpType.add,
                        )
                        nc.vector.tensor_add(dve_acc[:], dve_acc[:], part[:])
                    dve_chunks += 1

            # alpha: memset rides Pool after its add chain; out on idle SP ring
            alpha_t = opool.tile([BS, TX], f32)
            nc.gpsimd.memset(alpha_t[:], 1.0)
            nc.sync.dma_start(alpha_o[:], alpha_t[:])

            # merge partial accumulators, close the PSUM group, write out
            nc.gpsimd.tensor_add(dve_acc[:], dve_acc[:], pool_acc[:])
            nc.tensor.matmul(pacc[:], S[:], dve_acc[:], start=(mm_i == 0), stop=True)

            out_sb = opool.tile([BS, ENC], f32)
            nc.scalar.copy(out_sb[:], pacc[:])
            nc.scalar.dma_start(ctx_o[:], out_sb[:])

    nc.finalize()
    return nc


def _ones_blk():
    s = np.zeros((P, BS), dtype=np.float32)
    s[np.arange(P), np.arange(P) // (P // BS)] = 1.0
    return s


def kernel(a, h, coverage, Wmat, v, w, b):
    global LAST_RESULTS
    from concourse.bass_utils import run_bass_kernel_spmd

    if "nc" not in _CACHE:
        _CACHE["nc"] = _build_nc()
    nc = _CACHE["nc"]

    a = np.ascontiguousarray(np.asarray(a, dtype=np.float32))
    s = _ones_blk()
    in_maps = [
        {
            "a_shard": a[c * BS : (c + 1) * BS].reshape(P, TPP, ENC),
            "ones_blk": s,
        }
        for c in range(NCORES)
    ]

    res = run_bass_kernel_spmd(nc, in_maps, core_ids=list(range(NCORES)), trace=TRACE)
    LAST_RESULTS = res

    context = np.concatenate([r["ctx_out"] for r in res.results], axis=0)
    alpha = np.concatenate([r["alpha_out"] for r in res.results], axis=0)
    return context[:, None, :], alpha


# revision 6
# speedup vs baseline: 2.3164x; 1.0167x over previous
"""Trainium2 Bass kernel for nn_BahdanauAttention_17257178595902.

Reference math: softmax over a size-1 axis makes alpha identically 1.0,
so context[b, :] = sum_t a[b, t, :] and alpha = ones(B, Tx). The kernel
is a pure memory-bound reduction over `a` (128 x 400 x 512 f32, ~105 MB).

Sharding: data-parallel over batch B across 8 cores (16 rows each).
Per core the shard a[16, 400, 512] is viewed as [128, 50, 512]: SBUF
partition p holds 50 consecutive Tx-rows, all of local batch b = p // 8.

Per-core schedule (tuned against the CoreSim cost model, 59.4us -> 26.3us):
 - 11 uneven chunks (small first to warm PE early, small last to cut the
   tail), loads striped over the three DMA issue paths (SP HWDGE, ACT
   HWDGE, GpSimd SWDGE) and all issued upfront (bufs = n_chunks, ~132 KB
   per partition, no buffer stalls).
 - the Tx-sum is split three ways: PE (fp32 matmuls with a constant
   block-diagonal ones stationary S[128,16], accumulated in one PSUM
   group), DVE (strided tensor_reduce over t + adds into dve_acc), and
   GpSimd (tensor_add chain into pool_acc).
 - merge: pool_acc folded into dve_acc on DVE, then one final PE matmul
   closes the PSUM accumulation group; ACT copies PSUM->SBUF and DMAs
   the context out. alpha is a GpSimd memset-ones tile DMA'd out on the
   idle SP ring.
"""

import sys

for _p in ("/opt/trn_rl_repo",):
    if _p not in sys.path:
        sys.path.append(_p)

import numpy as np

B, TX, ENC = 128, 400, 512
NCORES = 8
BS = B // NCORES  # 16 local batch rows per core
P = 128  # SBUF partitions
TPP = BS * TX // P  # 50 Tx-rows per partition

# chunk sizes (t-rows), per-chunk (pe_k, pool_k) slice split (rest -> DVE),
# and DMA ring per chunk (0 = sync/SP HWDGE, 1 = scalar/ACT HWDGE, 2 = SWDGE)
CHUNKS = [2, 4, 5, 6, 6, 6, 6, 6, 5, 2, 2]
SPLIT = [(1, 1), (2, 0), (2, 1), (2, 2), (2, 3), (1, 2), (2, 2), (2, 2), (1, 2), (1, 1), (1, 1)]
RING = [0, 1, 2, 0, 1, 2, 0, 1, 2, 0, 1]

TRACE = False  # set by test harness to capture an NTFF profile
LAST_RESULTS = None  # BassKernelResults of the most recent run

_CACHE = {}


def _build_nc():
    import concourse.bacc as bacc
    import concourse.mybir as mybir
    from concourse.tile import TileContext

    f32 = mybir.dt.float32
    nc = bacc.Bacc("TRN2", target_bir_lowering=False)

    a = nc.dram_tensor("a_shard", [P, TPP, ENC], f32, kind="ExternalInput")
    s = nc.dram_tensor("ones_blk", [P, BS], f32, kind="ExternalInput")
    ctx_o = nc.dram_tensor("ctx_out", [BS, ENC], f32, kind="ExternalOutput")
    alpha_o = nc.dram_tensor("alpha_out", [BS, TX], f32, kind="ExternalOutput")

    with TileContext(nc) as tc:
        rings = [nc.sync, nc.scalar, nc.gpsimd]
        with (
            tc.tile_pool(name="io", bufs=len(CHUNKS)) as pool,
            tc.tile_pool(name="cst", bufs=1) as cpool,
            tc.tile_pool(name="red", bufs=3) as rpool,
            tc.tile_pool(name="accp", bufs=1) as apool,
            tc.tile_pool(name="psum", bufs=1, space="PSUM") as ppool,
            tc.tile_pool(name="outp", bufs=1) as opool,
        ):
            S = cpool.tile([P, BS], f32)
            nc.scalar.dma_start(S[:], s[:])

            # all chunk loads upfront; per-ring FIFO order = chunk order
            tiles = []
            t0 = 0
            maxc = max(CHUNKS)
            for c, tch in enumerate(CHUNKS):
                tl = pool.tile([P, maxc, ENC], f32, tag="io", name=f"io{c}")
                rings[RING[c]].dma_start(tl[:, :tch, :], a[:, t0 : t0 + tch, :])
                tiles.append(tl)
                t0 += tch

            pacc = ppool.tile([BS, ENC], f32)
            dve_acc = apool.tile([P, ENC], f32, name="dve_acc")
            pool_acc = apool.tile([P, ENC], f32, name="pool_acc")

            mm_i = 0
            dve_chunks = 0
            pool_slices = 0
            for c, (tch, (pe_k, pool_k)) in enumerate(zip(CHUNKS, SPLIT)):
                tl = tiles[c]
                for t in range(pe_k):
                    nc.tensor.matmul(
                        pacc[:], S[:], tl[:, t, :], start=(mm_i == 0), stop=False
                    )
                    mm_i += 1
                for t in range(pe_k, pe_k + pool_k):
                    if pool_slices == 0:
                        nc.gpsimd.tensor_copy(pool_acc[:], tl[:, t, :])
                    else:
                        nc.gpsimd.tensor_add(pool_acc[:], pool_acc[:], tl[:, t, :])
                    pool_slices += 1
                if tch - pe_k - pool_k > 0:
                    tview = tl[:, pe_k + pool_k : tch, :].rearrange("p t e -> p e t")
                    if dve_chunks == 0:
                        nc.vector.tensor_reduce(
                            dve_acc[:], tview,
                            axis=mybir.AxisListType.X, op=mybir.AluOpType.add,
                        )
                    else:
                        part = rpool.tile([P, ENC], f32, tag="part", name="part")
                        nc.vector.tensor_reduce(
                            part[:], tview,
                            axis=mybir.AxisListType.X, op=mybir.AluOpType.add,
                        )
                        nc.vector.tensor_add(dve_acc[:], dve_acc[:], part[:])
                    dve_chunks += 1

            # alpha: memset rides Pool after its add chain; out on idle SP ring
            alpha_t = opool.tile([BS, TX], f32)
            nc.gpsimd.memset(alpha_t[:], 1.0)
            nc.sync.dma_start(alpha_o[:], alpha_t[:])

            # merge partial accumulators, close the PSUM group, write out
            nc.gpsimd.tensor_add(dve_acc[:], dve_acc[:], pool_acc[:])
            nc.tensor.matmul(pacc[:], S[:], dve_acc[:], start=(mm_i == 0), stop=True)

            out_sb = opool.tile([BS, ENC], f32)
            nc.scalar.copy(out_sb[:], pacc[:])
            nc.scalar.dma_start(ctx_o[:], out_sb[:])

    nc.finalize()
    return nc


def _ones_blk():
    s = np.zeros((P, BS), dtype=np.float32)
    s[np.arange(P), np.arange(P) // (P // BS)] = 1.0
    return s


def kernel(a, h, coverage, Wmat, v, w, b):
    global LAST_RESULTS
    from concourse.bass_utils import run_bass_kernel_spmd

    if "nc" not in _CACHE:
        _CACHE["nc"] = _build_nc()
    nc = _CACHE["nc"]

    a = np.ascontiguousarray(np.asarray(a, dtype=np.float32))
    s = _ones_blk()
    in_maps = [
        {
            "a_shard": a[c * BS : (c + 1) * BS].reshape(P, TPP, ENC),
            "ones_blk": s,
        }
        for c in range(NCORES)
    ]

    res = run_bass_kernel_spmd(nc, in_maps, core_ids=list(range(NCORES)), trace=TRACE)
    LAST_RESULTS = res

    context = np.concatenate([r["ctx_out"] for r in res.results], axis=0)
    alpha = np.concatenate([r["alpha_out"] for r in res.results], axis=0)
    return context[:, None, :], alpha


# revision 7
# speedup vs baseline: 2.5035x; 1.0808x over previous
"""Trainium2 Bass kernel for nn_BahdanauAttention_17257178595902.

Reference math: softmax over a size-1 axis makes alpha identically 1.0,
so context[b, :] = sum_t a[b, t, :] and alpha = ones(B, Tx). The kernel
is a pure memory-bound reduction over `a` (128 x 400 x 512 f32, ~105 MB).

Sharding: data-parallel over batch B across 8 cores (16 rows each).
Per core the shard a[16, 400, 512] is viewed as [128, 50, 512]: SBUF
partition p holds 50 consecutive Tx-rows, all of local batch b = p // 8.

Per-core schedule (tuned against the CoreSim cost model, 59.4us -> 26.3us):
 - 11 uneven chunks (small first to warm PE early, small last to cut the
   tail), loads striped over the three DMA issue paths (SP HWDGE, ACT
   HWDGE, GpSimd SWDGE) and all issued upfront (bufs = n_chunks, ~132 KB
   per partition, no buffer stalls).
 - the Tx-sum is split three ways: PE (fp32 matmuls with a constant
   block-diagonal ones stationary S[128,16], accumulated in one PSUM
   group), DVE (strided tensor_reduce over t + adds into dve_acc), and
   GpSimd (tensor_add chain into pool_acc).
 - merge: pool_acc folded into dve_acc on DVE, then one final PE matmul
   closes the PSUM accumulation group; ACT copies PSUM->SBUF and DMAs
   the context out. alpha is a GpSimd memset-ones tile DMA'd out on the
   idle SP ring.
"""

import sys

for _p in ("/opt/trn_rl_repo",):
    if _p not in sys.path:
        sys.path.append(_p)

import numpy as np

B, TX, ENC = 128, 400, 512
NCORES = 8
BS = B // NCORES  # 16 local batch rows per core
P = 128  # SBUF partitions
TPP = BS * TX // P  # 50 Tx-rows per partition

# chunk sizes (t-rows), per-chunk (pe_k, pool_k) slice split (rest -> DVE),
# and DMA ring per chunk (0 = sync/SP HWDGE, 1 = scalar/ACT HWDGE, 2 = SWDGE)
CHUNKS = [2, 4, 5, 7, 5, 6, 6, 6, 4, 2, 3]
SPLIT = [(1, 1), (2, 1), (2, 1), (2, 3), (2, 3), (1, 2), (1, 3), (2, 2), (0, 4), (1, 1), (1, 0)]
RING = [0, 1, 2, 0, 1, 2, 0, 1, 0, 0, 1]

TRACE = False  # set by test harness to capture an NTFF profile
LAST_RESULTS = None  # BassKernelResults of the most recent run

_CACHE = {}


def _build_nc():
    import concourse.bacc as bacc
    import concourse.mybir as mybir
    from concourse.tile import TileContext

    f32 = mybir.dt.float32
    nc = bacc.Bacc("TRN2", target_bir_lowering=False)

    a = nc.dram_tensor("a_shard", [P, TPP, ENC], f32, kind="ExternalInput")
    s = nc.dram_tensor("ones_blk", [P, BS], f32, kind="ExternalInput")
    ctx_o = nc.dram_tensor("ctx_out", [BS, ENC], f32, kind="ExternalOutput")
    alpha_o = nc.dram_tensor("alpha_out", [BS, TX], f32, kind="ExternalOutput")

    with TileContext(nc) as tc:
        rings = [nc.sync, nc.scalar, nc.gpsimd]
        with (
            tc.tile_pool(name="io", bufs=len(CHUNKS)) as pool,
            tc.tile_pool(name="cst", bufs=1) as cpool,
            tc.tile_pool(name="red", bufs=3) as rpool,
            tc.tile_pool(name="accp", bufs=1) as apool,
            tc.tile_pool(name="psum", bufs=1, space="PSUM") as ppool,
            tc.tile_pool(name="outp", bufs=1) as opool,
        ):
            S = cpool.tile([P, BS], f32)
            nc.scalar.dma_start(S[:], s[:])

            # all chunk loads upfront; per-ring FIFO order = chunk order
            tiles = []
            t0 = 0
            maxc = max(CHUNKS)
            for c, tch in enumerate(CHUNKS):
                tl = pool.tile([P, maxc, ENC], f32, tag="io", name=f"io{c}")
                rings[RING[c]].dma_start(tl[:, :tch, :], a[:, t0 : t0 + tch, :])
                tiles.append(tl)
                t0 += tch

            pacc = ppool.tile([BS, ENC], f32)
            dve_acc = apool.tile([P, ENC], f32, name="dve_acc")
            pool_acc = apool.tile([P, ENC], f32, name="pool_acc")

            mm_i = 0
            dve_chunks = 0
            pool_slices = 0
            for c, (tch, (pe_k, pool_k)) in enumerate(zip(CHUNKS, SPLIT)):
                tl = tiles[c]
                for t in range(pe_k):
                    nc.tensor.matmul(
                        pacc[:], S[:], tl[:, t, :], start=(mm_i == 0), stop=False
                    )
                    mm_i += 1
                for t in range(pe_k, pe_k + pool_k):
                    if pool_slices == 0:
                        nc.gpsimd.tensor_copy(pool_acc[:], tl[:, t, :])
                    else:
                        nc.gpsimd.tensor_add(pool_acc[:], pool_acc[:], tl[:, t, :])
                    pool_slices += 1
                if tch - pe_k - pool_k > 0:
                    tview = tl[:, pe_k + pool_k : tch, :].rearrange("p t e -> p e t")
                    if dve_chunks == 0:
                        nc.vector.tensor_reduce(
                            dve_acc[:], tview,
                            axis=mybir.AxisListType.X, op=mybir.AluOpType.add,
                        )
                    else:
                        part = rpool.tile([P, ENC], f32, tag="part", name="part")
                        nc.vector.tensor_reduce(
                            part[:], tview,
                            axis=mybir.AxisListType.X, op=mybir.AluOpType.add,
                        )
                        nc.vector.tensor_add(dve_acc[:], dve_acc[:], part[:])
                    dve_chunks += 1

            # alpha: memset rides Pool after its add chain; out on idle SP ring
            alpha_t = opool.tile([BS, TX], f32)
            nc.gpsimd.memset(alpha_t[:], 1.0)
            nc.sync.dma_start(alpha_o[:], alpha_t[:])

            # merge partial accumulators, close the PSUM group, write out
            nc.gpsimd.tensor_add(dve_acc[:], dve_acc[:], pool_acc[:])
            nc.tensor.matmul(pacc[:], S[:], dve_acc[:], start=(mm_i == 0), stop=True)

            out_sb = opool.tile([BS, ENC], f32)
            nc.scalar.copy(out_sb[:], pacc[:])
            nc.scalar.dma_start(ctx_o[:], out_sb[:])

    nc.finalize()
    return nc


def _ones_blk():
    s = np.zeros((P, BS), dtype=np.float32)
    s[np.arange(P), np.arange(P) // (P // BS)] = 1.0
    return s


def kernel(a, h, coverage, Wmat, v, w, b):
    global LAST_RESULTS
    from concourse.bass_utils import run_bass_kernel_spmd

    if "nc" not in _CACHE:
        _CACHE["nc"] = _build_nc()
    nc = _CACHE["nc"]

    a = np.ascontiguousarray(np.asarray(a, dtype=np.float32))
    s = _ones_blk()
    in_maps = [
        {
            "a_shard": a[c * BS : (c + 1) * BS].reshape(P, TPP, ENC),
            "ones_blk": s,
        }
        for c in range(NCORES)
    ]

    res = run_bass_kernel_spmd(nc, in_maps, core_ids=list(range(NCORES)), trace=TRACE)
    LAST_RESULTS = res

    context = np.concatenate([r["ctx_out"] for r in res.results], axis=0)
    alpha = np.concatenate([r["alpha_out"] for r in res.results], axis=0)
    return context[:, None, :], alpha


# revision 8
# speedup vs baseline: 2.5246x; 1.0084x over previous
"""Trainium2 Bass kernel for nn_BahdanauAttention_17257178595902.

Reference math: softmax over a size-1 axis makes alpha identically 1.0,
so context[b, :] = sum_t a[b, t, :] and alpha = ones(B, Tx). The kernel
is a pure memory-bound reduction over `a` (128 x 400 x 512 f32, ~105 MB).

Sharding: data-parallel over batch B across 8 cores (16 rows each).
Per core the shard a[16, 400, 512] is viewed as [128, 50, 512]: SBUF
partition p holds 50 consecutive Tx-rows, all of local batch b = p // 8.

Per-core schedule (tuned against the CoreSim cost model, 59.4us -> 26.3us):
 - 11 uneven chunks (small first to warm PE early, small last to cut the
   tail), loads striped over the three DMA issue paths (SP HWDGE, ACT
   HWDGE, GpSimd SWDGE) and all issued upfront (bufs = n_chunks, ~132 KB
   per partition, no buffer stalls).
 - the Tx-sum is split three ways: PE (fp32 matmuls with a constant
   block-diagonal ones stationary S[128,16], accumulated in one PSUM
   group), DVE (strided tensor_reduce over t + adds into dve_acc), and
   GpSimd (tensor_add chain into pool_acc).
 - merge: pool_acc folded into dve_acc on DVE, then one final PE matmul
   closes the PSUM accumulation group; ACT copies PSUM->SBUF and DMAs
   the context out. alpha is a GpSimd memset-ones tile DMA'd out on the
   idle SP ring.
"""

import sys

for _p in ("/opt/trn_rl_repo",):
    if _p not in sys.path:
        sys.path.append(_p)

import numpy as np

B, TX, ENC = 128, 400, 512
NCORES = 8
BS = B // NCORES  # 16 local batch rows per core
P = 128  # SBUF partitions
TPP = BS * TX // P  # 50 Tx-rows per partition

# chunk sizes (t-rows), per-chunk (pe_k, pool_k) slice split (rest -> DVE),
# and DMA ring per chunk (0 = sync/SP HWDGE, 1 = scalar/ACT HWDGE, 2 = SWDGE)
CHUNKS = [4, 4, 5, 7, 5, 6, 4, 6, 4, 2, 3]
SPLIT = [(1, 0), (2, 0), (2, 1), (2, 3), (1, 4), (1, 2), (1, 3), (2, 2), (0, 4), (1, 1), (0, 1)]
RING = [0, 1, 2, 0, 1, 2, 0, 1, 0, 0, 1]

TRACE = False  # set by test harness to capture an NTFF profile
LAST_RESULTS = None  # BassKernelResults of the most recent run

_CACHE = {}


def _build_nc():
    import concourse.bacc as bacc
    import concourse.mybir as mybir
    from concourse.tile import TileContext

    f32 = mybir.dt.float32
    nc = bacc.Bacc("TRN2", target_bir_lowering=False)

    a = nc.dram_tensor("a_shard", [P, TPP, ENC], f32, kind="ExternalInput")
    s = nc.dram_tensor("ones_blk", [P, BS], f32, kind="ExternalInput")
    ctx_o = nc.dram_tensor("ctx_out", [BS, ENC], f32, kind="ExternalOutput")
    alpha_o = nc.dram_tensor("alpha_out", [BS, TX], f32, kind="ExternalOutput")

    with TileContext(nc) as tc:
        rings = [nc.sync, nc.scalar, nc.gpsimd]
        with (
            tc.tile_pool(name="io", bufs=len(CHUNKS)) as pool,
            tc.tile_pool(name="cst", bufs=1) as cpool,
            tc.tile_pool(name="red", bufs=3) as rpool,
            tc.tile_pool(name="accp", bufs=1) as apool,
            tc.tile_pool(name="psum", bufs=1, space="PSUM") as ppool,
            tc.tile_pool(name="outp", bufs=1) as opool,
        ):
            S = cpool.tile([P, BS], f32)
            nc.scalar.dma_start(S[:], s[:])

            # all chunk loads upfront; per-ring FIFO order = chunk order
            tiles = []
            t0 = 0
            maxc = max(CHUNKS)
            for c, tch in enumerate(CHUNKS):
                tl = pool.tile([P, maxc, ENC], f32, tag="io", name=f"io{c}")
                rings[RING[c]].dma_start(tl[:, :tch, :], a[:, t0 : t0 + tch, :])
                tiles.append(tl)
                t0 += tch

            pacc = ppool.tile([BS, ENC], f32)
            dve_acc = apool.tile([P, ENC], f32, name="dve_acc")
            pool_acc = apool.tile([P, ENC], f32, name="pool_acc")

            mm_i = 0
            dve_chunks = 0
            pool_slices = 0
            for c, (tch, (pe_k, pool_k)) in enumerate(zip(CHUNKS, SPLIT)):
                tl = tiles[c]
                for t in range(pe_k):
                    nc.tensor.matmul(
                        pacc[:], S[:], tl[:, t, :], start=(mm_i == 0), stop=False
                    )
                    mm_i += 1
                for t in range(pe_k, pe_k + pool_k):
                    if pool_slices == 0:
                        nc.gpsimd.tensor_copy(pool_acc[:], tl[:, t, :])
                    else:
                        nc.gpsimd.tensor_add(pool_acc[:], pool_acc[:], tl[:, t, :])
                    pool_slices += 1
                if tch - pe_k - pool_k > 0:
                    tview = tl[:, pe_k + pool_k : tch, :].rearrange("p t e -> p e t")
                    if dve_chunks == 0:
                        nc.vector.tensor_reduce(
                            dve_acc[:], tview,
                            axis=mybir.AxisListType.X, op=mybir.AluOpType.add,
                        )
                    else:
                        part = rpool.tile([P, ENC], f32, tag="part", name="part")
                        nc.vector.tensor_reduce(
                            part[:], tview,
                            axis=mybir.AxisListType.X, op=mybir.AluOpType.add,
                        )
                        nc.vector.tensor_add(dve_acc[:], dve_acc[:], part[:])
                    dve_chunks += 1

            # alpha: memset rides Pool after its add chain; out on idle SP ring
            alpha_t = opool.tile([BS, TX], f32)
            nc.gpsimd.memset(alpha_t[:], 1.0)
            nc.sync.dma_start(alpha_o[:], alpha_t[:])

            # merge partial accumulators, close the PSUM group, write out
            nc.gpsimd.tensor_add(dve_acc[:], dve_acc[:], pool_acc[:])
            nc.tensor.matmul(pacc[:], S[:], dve_acc[:], start=(mm_i == 0), stop=True)

            out_sb = opool.tile([BS, ENC], f32)
            nc.vector.tensor_copy(out_sb[:], pacc[:])
            nc.scalar.dma_start(ctx_o[:], out_sb[:])

    nc.finalize()
    return nc


def _ones_blk():
    s = np.zeros((P, BS), dtype=np.float32)
    s[np.arange(P), np.arange(P) // (P // BS)] = 1.0
    return s


def kernel(a, h, coverage, Wmat, v, w, b):
    global LAST_RESULTS
    from concourse.bass_utils import run_bass_kernel_spmd

    if "nc" not in _CACHE:
        _CACHE["nc"] = _build_nc()
    nc = _CACHE["nc"]

    a = np.ascontiguousarray(np.asarray(a, dtype=np.float32))
    s = _ones_blk()
    in_maps = [
        {
            "a_shard": a[c * BS : (c + 1) * BS].reshape(P, TPP, ENC),
            "ones_blk": s,
        }
        for c in range(NCORES)
    ]

    res = run_bass_kernel_spmd(nc, in_maps, core_ids=list(range(NCORES)), trace=TRACE)
    LAST_RESULTS = res

    context = np.concatenate([r["ctx_out"] for r in res.results], axis=0)
    alpha = np.concatenate([r["alpha_out"] for r in res.results], axis=0)
    return context[:, None, :], alpha


# revision 9
# speedup vs baseline: 2.5334x; 1.0035x over previous
"""Trainium2 Bass kernel for nn_BahdanauAttention_17257178595902.

Reference math: softmax over a size-1 axis makes alpha identically 1.0,
so context[b, :] = sum_t a[b, t, :] and alpha = ones(B, Tx). The kernel
is a pure memory-bound reduction over `a` (128 x 400 x 512 f32, ~105 MB).

Sharding: data-parallel over batch B across 8 cores (16 rows each).
Per core the shard a[16, 400, 512] is viewed as [128, 50, 512]: SBUF
partition p holds 50 consecutive Tx-rows, all of local batch b = p // 8.

Per-core schedule (tuned against the CoreSim cost model, 59.4us -> 26.3us):
 - 11 uneven chunks (small first to warm PE early, small last to cut the
   tail), loads striped over the three DMA issue paths (SP HWDGE, ACT
   HWDGE, GpSimd SWDGE) and all issued upfront (bufs = n_chunks, ~132 KB
   per partition, no buffer stalls).
 - the Tx-sum is split three ways: PE (fp32 matmuls with a constant
   block-diagonal ones stationary S[128,16], accumulated in one PSUM
   group), DVE (strided tensor_reduce over t + adds into dve_acc), and
   GpSimd (tensor_add chain into pool_acc).
 - merge: pool_acc folded into dve_acc on DVE, then one final PE matmul
   closes the PSUM accumulation group; ACT copies PSUM->SBUF and DMAs
   the context out. alpha is a GpSimd memset-ones tile DMA'd out on the
   idle SP ring.
"""

import sys

for _p in ("/opt/trn_rl_repo",):
    if _p not in sys.path:
        sys.path.append(_p)

import numpy as np

B, TX, ENC = 128, 400, 512
NCORES = 8
BS = B // NCORES  # 16 local batch rows per core
P = 128  # SBUF partitions
TPP = BS * TX // P  # 50 Tx-rows per partition

# chunk sizes (t-rows), per-chunk (pe_k, pool_k) slice split (rest -> DVE),
# and DMA ring per chunk (0 = sync/SP HWDGE, 1 = scalar/ACT HWDGE, 2 = SWDGE)
CHUNKS = [3, 4, 5, 7, 6, 7, 4, 5, 4, 2, 3]
SPLIT = [(1, 0), (2, 0), (2, 1), (2, 3), (1, 2), (1, 2), (1, 3), (2, 2), (0, 4), (2, 0), (0, 2)]
RING = [0, 1, 2, 0, 1, 2, 0, 1, 0, 0, 1]

TRACE = False  # set by test harness to capture an NTFF profile
LAST_RESULTS = None  # BassKernelResults of the most recent run

_CACHE = {}


def _build_nc():
    import concourse.bacc as bacc
    import concourse.mybir as mybir
    from concourse.tile import TileContext

    f32 = mybir.dt.float32
    nc = bacc.Bacc("TRN2", target_bir_lowering=False)

    a = nc.dram_tensor("a_shard", [P, TPP, ENC], f32, kind="ExternalInput")
    s = nc.dram_tensor("ones_blk", [P, BS], f32, kind="ExternalInput")
    ctx_o = nc.dram_tensor("ctx_out", [BS, ENC], f32, kind="ExternalOutput")
    alpha_o = nc.dram_tensor("alpha_out", [BS, TX], f32, kind="ExternalOutput")

    with TileContext(nc) as tc:
        rings = [nc.sync, nc.scalar, nc.gpsimd]
        with (
            tc.tile_pool(name="io", bufs=len(CHUNKS)) as pool,
            tc.tile_pool(name="cst", bufs=1) as cpool,
            tc.tile_pool(name="red", bufs=3) as rpool,
            tc.tile_pool(name="accp", bufs=1) as apool,
            tc.tile_pool(name="psum", bufs=1, space="PSUM") as ppool,
            tc.tile_pool(name="outp", bufs=1) as opool,
        ):
            S = cpool.tile([P, BS], f32)
            nc.scalar.dma_start(S[:], s[:])

            # all chunk loads upfront; per-ring FIFO order = chunk order
            tiles = []
            t0 = 0
            maxc = max(CHUNKS)
            for c, tch in enumerate(CHUNKS):
                tl = pool.tile([P, maxc, ENC], f32, tag="io", name=f"io{c}")
                rings[RING[c]].dma_start(tl[:, :tch, :], a[:, t0 : t0 + tch, :])
                tiles.append(tl)
                t0 += tch

            pacc = ppool.tile([BS, ENC], f32)
            dve_acc = apool.tile([P, ENC], f32, name="dve_acc")
            pool_acc = apool.tile([P, ENC], f32, name="pool_acc")

            mm_i = 0
            dve_chunks = 0
            pool_slices = 0
            for c, (tch, (pe_k, pool_k)) in enumerate(zip(CHUNKS, SPLIT)):
                tl = tiles[c]
                for t in range(pe_k):
                    nc.tensor.matmul(
                        pacc[:], S[:], tl[:, t, :], start=(mm_i == 0), stop=False
                    )
                    mm_i += 1
                for t in range(pe_k, pe_k + pool_k):
                    if pool_slices == 0:
                        nc.gpsimd.tensor_copy(pool_acc[:], tl[:, t, :])
                    else:
                        nc.gpsimd.tensor_add(pool_acc[:], pool_acc[:], tl[:, t, :])
                    pool_slices += 1
                if tch - pe_k - pool_k > 0:
                    tview = tl[:, pe_k + pool_k : tch, :].rearrange("p t e -> p e t")
                    if dve_chunks == 0:
                        nc.vector.tensor_reduce(
                            dve_acc[:], tview,
                            axis=mybir.AxisListType.X, op=mybir.AluOpType.add,
                        )
                    else:
                        part = rpool.tile([P, ENC], f32, tag="part", name="part")
                        nc.vector.tensor_reduce(
                            part[:], tview,
                            axis=mybir.AxisListType.X, op=mybir.AluOpType.add,
                        )
                        nc.vector.tensor_add(dve_acc[:], dve_acc[:], part[:])
                    dve_chunks += 1

            # alpha: memset rides Pool after its add chain; out on idle SP ring
            alpha_t = opool.tile([BS, TX], f32)
            nc.gpsimd.memset(alpha_t[:], 1.0)
            nc.sync.dma_start(alpha_o[:], alpha_t[:])

            # merge partial accumulators, close the PSUM group, write out
            nc.gpsimd.tensor_add(dve_acc[:], dve_acc[:], pool_acc[:])
            nc.tensor.matmul(pacc[:], S[:], dve_acc[:], start=(mm_i == 0), stop=True)

            out_sb = opool.tile([BS, ENC], f32)
            nc.scalar.copy(out_sb[:], pacc[:])
            nc.scalar.dma_start(ctx_o[:], out_sb[:])

    nc.finalize()
    return nc


def _ones_blk():
    s = np.zeros((P, BS), dtype=np.float32)
    s[np.arange(P), np.arange(P) // (P // BS)] = 1.0
    return s


def kernel(a, h, coverage, Wmat, v, w, b):
    global LAST_RESULTS
    from concourse.bass_utils import run_bass_kernel_spmd

    if "nc" not in _CACHE:
        _CACHE["nc"] = _build_nc()
    nc = _CACHE["nc"]

    a = np.ascontiguousarray(np.asarray(a, dtype=np.float32))
    s = _ones_blk()
    in_maps = [
        {
            "a_shard": a[c * BS : (c + 1) * BS].reshape(P, TPP, ENC),
            "ones_blk": s,
        }
        for c in range(NCORES)
    ]

    res = run_bass_kernel_spmd(nc, in_maps, core_ids=list(range(NCORES)), trace=TRACE)
    LAST_RESULTS = res

    context = np.concatenate([r["ctx_out"] for r in res.results], axis=0)
    alpha = np.concatenate([r["alpha_out"] for r in res.results], axis=0)
    return context[:, None, :], alpha


# revision 10
# speedup vs baseline: 2.5693x; 1.0142x over previous
"""Trainium2 Bass kernel for nn_BahdanauAttention_17257178595902.

Reference math: softmax over a size-1 axis makes alpha identically 1.0,
so context[b, :] = sum_t a[b, t, :] and alpha = ones(B, Tx). The kernel
is a pure memory-bound reduction over `a` (128 x 400 x 512 f32, ~105 MB).

Sharding: data-parallel over batch B across 8 cores (16 rows each).
Per core the shard a[16, 400, 512] is viewed as [128, 50, 512]: SBUF
partition p holds 50 consecutive Tx-rows, all of local batch b = p // 8.

Per-core schedule (tuned against the CoreSim cost model, 59.4us -> 26.3us):
 - 11 uneven chunks (small first to warm PE early, small last to cut the
   tail), loads striped over the three DMA issue paths (SP HWDGE, ACT
   HWDGE, GpSimd SWDGE) and all issued upfront (bufs = n_chunks, ~132 KB
   per partition, no buffer stalls).
 - the Tx-sum is split three ways: PE (fp32 matmuls with a constant
   block-diagonal ones stationary S[128,16], accumulated in one PSUM
   group), DVE (strided tensor_reduce over t + adds into dve_acc), and
   GpSimd (tensor_add chain into pool_acc).
 - merge: pool_acc folded into dve_acc on DVE, then one final PE matmul
   closes the PSUM accumulation group; ACT copies PSUM->SBUF and DMAs
   the context out. alpha is a GpSimd memset-ones tile DMA'd out on the
   idle SP ring.
"""

import sys

for _p in ("/opt/trn_rl_repo",):
    if _p not in sys.path:
        sys.path.append(_p)

import numpy as np

B, TX, ENC = 128, 400, 512
NCORES = 8
BS = B // NCORES  # 16 local batch rows per core
P = 128  # SBUF partitions
TPP = BS * TX // P  # 50 Tx-rows per partition

# chunk sizes (t-rows), per-chunk (pe_k, pool_k) slice split (rest -> DVE),
# and DMA ring per chunk (0 = sync/SP HWDGE, 1 = scalar/ACT HWDGE, 2 = SWDGE)
CHUNKS = [3, 5, 7, 8, 6, 4, 3, 5, 4, 2, 3]
SPLIT = [(1, 0), (2, 0), (3, 0), (1, 3), (0, 4), (0, 3), (1, 1), (3, 2), (0, 4), (1, 1), (1, 2)]
RING = [0, 1, 2, 0, 1, 2, 0, 1, 0, 0, 1]

TRACE = False  # set by test harness to capture an NTFF profile
LAST_RESULTS = None  # BassKernelResults of the most recent run

_CACHE = {}


def _build_nc():
    import concourse.bacc as bacc
    import concourse.mybir as mybir
    from concourse.tile import TileContext

    f32 = mybir.dt.float32
    nc = bacc.Bacc("TRN2", target_bir_lowering=False)

    a = nc.dram_tensor("a_shard", [P, TPP, ENC], f32, kind="ExternalInput")
    s = nc.dram_tensor("ones_blk", [P, BS], f32, kind="ExternalInput")
    ctx_o = nc.dram_tensor("ctx_out", [BS, ENC], f32, kind="ExternalOutput")
    alpha_o = nc.dram_tensor("alpha_out", [BS, TX], f32, kind="ExternalOutput")

    with TileContext(nc) as tc:
        rings = [nc.sync, nc.scalar, nc.gpsimd]
        with (
            tc.tile_pool(name="io", bufs=len(CHUNKS)) as pool,
            tc.tile_pool(name="cst", bufs=1) as cpool,
            tc.tile_pool(name="red", bufs=3) as rpool,
            tc.tile_pool(name="accp", bufs=1) as apool,
            tc.tile_pool(name="psum", bufs=1, space="PSUM") as ppool,
            tc.tile_pool(name="outp", bufs=1) as opool,
        ):
            S = cpool.tile([P, BS], f32)
            nc.scalar.dma_start(S[:], s[:])

            # all chunk loads upfront; per-ring FIFO order = chunk order
            tiles = []
            t0 = 0
            maxc = max(CHUNKS)
            for c, tch in enumerate(CHUNKS):
                tl = pool.tile([P, maxc, ENC], f32, tag="io", name=f"io{c}")
                rings[RING[c]].dma_start(tl[:, :tch, :], a[:, t0 : t0 + tch, :])
                tiles.append(tl)
                t0 += tch

            pacc = ppool.tile([BS, ENC], f32)
            dve_acc = apool.tile([P, ENC], f32, name="dve_acc")
            pool_acc = apool.tile([P, ENC], f32, name="pool_acc")

            mm_i = 0
            dve_chunks = 0
            pool_slices = 0
            for c, (tch, (pe_k, pool_k)) in enumerate(zip(CHUNKS, SPLIT)):
                tl = tiles[c]
                for t in range(pe_k):
                    nc.tensor.matmul(
                        pacc[:], S[:], tl[:, t, :], start=(mm_i == 0), stop=False
                    )
                    mm_i += 1
                for t in range(pe_k, pe_k + pool_k):
                    if pool_slices == 0:
                        nc.gpsimd.tensor_copy(pool_acc[:], tl[:, t, :])
                    else:
                        nc.gpsimd.tensor_add(pool_acc[:], pool_acc[:], tl[:, t, :])
                    pool_slices += 1
                if tch - pe_k - pool_k > 0:
                    tview = tl[:, pe_k + pool_k : tch, :].rearrange("p t e -> p e t")
                    if dve_chunks == 0:
                        nc.vector.tensor_reduce(
                            dve_acc[:], tview,
                            axis=mybir.AxisListType.X, op=mybir.AluOpType.add,
                        )
                    else:
                        part = rpool.tile([P, ENC], f32, tag="part", name="part")
                        nc.vector.tensor_reduce(
                            part[:], tview,
                            axis=mybir.AxisListType.X, op=mybir.AluOpType.add,
                        )
                        nc.vector.tensor_add(dve_acc[:], dve_acc[:], part[:])
                    dve_chunks += 1

            # alpha: memset rides Pool after its add chain; out on idle SP ring
            alpha_t = opool.tile([BS, TX], f32)
            nc.gpsimd.memset(alpha_t[:], 1.0)
            nc.sync.dma_start(alpha_o[:], alpha_t[:])

            # merge partial accumulators, close the PSUM group, write out
            nc.gpsimd.tensor_add(dve_acc[:], dve_acc[:], pool_acc[:])
            nc.tensor.matmul(pacc[:], S[:], dve_acc[:], start=(mm_i == 0), stop=True)

            out_sb = opool.tile([BS, ENC], f32)
            nc.vector.tensor_copy(out_sb[:], pacc[:])
            nc.scalar.dma_start(ctx_o[:], out_sb[:])

    nc.finalize()
    return nc


def _ones_blk():
    s = np.zeros((P, BS), dtype=np.float32)
    s[np.arange(P), np.arange(P) // (P // BS)] = 1.0
    return s


def kernel(a, h, coverage, Wmat, v, w, b):
    global LAST_RESULTS
    from concourse.bass_utils import run_bass_kernel_spmd

    if "nc" not in _CACHE:
        _CACHE["nc"] = _build_nc()
    nc = _CACHE["nc"]

    a = np.ascontiguousarray(np.asarray(a, dtype=np.float32))
    s = _ones_blk()
    in_maps = [
        {
            "a_shard": a[c * BS : (c + 1) * BS].reshape(P, TPP, ENC),
            "ones_blk": s,
        }
        for c in range(NCORES)
    ]

    res = run_bass_kernel_spmd(nc, in_maps, core_ids=list(range(NCORES)), trace=TRACE)
    LAST_RESULTS = res

    context = np.concatenate([r["ctx_out"] for r in res.results], axis=0)
    alpha = np.concatenate([r["alpha_out"] for r in res.results], axis=0)
    return context[:, None, :], alpha
